# revision 78
# baseline (speedup 1.0000x reference)
"""Trainium2 Bass kernel for nn_DirectionalWedgeBias.

Computes, per (batch b, head h):
    v      = x[b].reshape(T, H, Dh)[:, h, :]          # [T, Dh]
    v_hat  = v / max(||v||_2, eps)  (row-wise)
    S      = A[h] - A[h]^T                            # [Dh, Dh]
    wedge  = (v_hat @ S) @ v_hat^T                    # [T, T]

Full shapes: x [2, 2048, 1024] f32, A [16, 64, 64] f32 -> out [2, 16, 2048, 2048] f32.

Sharding: 32 independent (b, h) pairs split 4-per-core across 8 NeuronCores
(data + head parallel; the tiny skew-symmetric S is replicated/sliced with the
heads). Host pre-slices x into per-core [4, T, Dh] blocks, forms S = A - A^T,
and re-stacks the per-core [4, T, T] results.

Per-core dataflow (Tile framework), designed around the v1 CoreSim cost model
and the walrus BIR constraints:

  - All wedge math runs in bf16 (v_hat, S, SvT; rel err ~4.7e-3 vs the 2e-2
    budget): bf16 matmuls/transposes run 1 cyc/row on the PE and bf16 SBUF
    elementwise ops hit the DVE/engine fast paths.
  - Interleaved m-blocks: the wedge row-block m of row-half ht uses
    lhsT = svt[:, m::8] within that half, so PSUM partition q of block m
    holds output row t = 1024*ht + q*8 + m.  A (half, col-group) sub-block
    stages [128 part, 8, 512] in SBUF whose flat (partition, chunk, col)
    order IS row-major DRAM order: the store's DRAM-side AP balances to
    [[rows, 1024], [1, 1], [1, 512]], whose free size (and hence DMA queue
    cost) is 512 elems -> ~0.8 us per 2 MiB store instead of the ~25 us a
    [128, ...]-leading AP would cost.  The staging tile keeps a 516-elem
    chunk stride so its free dims cannot re-merge during AP balancing.
  - PSUM->SBUF evacuation is the critical path: walrus forbids GPSIMD from
    touching PSUM, so only ACT (0.83 ns/elem + 185 ns/inst) and DVE
    (1.04 ns/elem + 125 ns/inst; 0.52 for all-2-byte ops) can drain the
    64 MiB of wedge PSUM.  A static greedy balancer splits every
    evacuation copy between them (~1.85 elems/ns combined -> ~80 us busy
    each; the PE's 61 us of matmuls hides under it).
  - Everything SBUF-only rides the otherwise-idle gpsimd queue: the x->bf16
    cast, the square, a pairwise-tree row-sum (replacing the DVE-only
    reduce), the 1/||v|| broadcast multiply, and the vT partition-half
    duplication.  SP carries all input loads and output stores.
  - A-phase packing: all 16 transposes of a pair write one [64, 2048] bf16
    PSUM view (one 2x DVE evacuation), gpsimd then mirrors vT onto
    partitions 64-127 so every wedge matmul finds lhsT and rhs on the same
    partition base; the 4 Sv matmuls pack into one [128, 1024] f32 slot
    (partition half = t-half, column half = group parity) for a single
    evacuation into the packed SvT.
  - PSUM: one shared 4-slot ring of [128, 1024] (2 banks each) serves wedge
    tiles, the transpose view and the Sv pack; the PE runs up to ~1.7 us
    ahead of the evacuations.  PE p-state: a warmup matmul starts the ramp
    clock during the fill (pe_busy_start never resets on the sub-us stalls
    this schedule produces, verified against the cost model).
  - Software pipelining: pair p+1's load/normalize (2 slices) and
    transpose/Sv (2 slices) are emitted between pair p's first four wedge
    sub-blocks; the drain splits the final evacuation ACT/DVE.
  - walrus encodes at most ONE semaphore wait on most instructions (and two
    on EventSemaphore), so `_spill_waits` post-processes the Tile-scheduled
    BIR, hoisting excess waits onto preceding same-engine EventSemaphores
    (sequencers run in order, so this is semantics-preserving).

Cost-model (CoreSim) per-core time: ~88.9 us vs the 121.5 us baseline
(engine busy: ACT ~79 us, DVE ~78 us, PE ~61 us, SP ~36 us, gpsimd ~22 us;
the fill pair additionally stages its vT/SvT evacuations per half inside the
same PSUM slot acquisitions so the first wedge block starts ~2 us sooner);
verified on the 8 NeuronCores at rel err 4.73e-3.  The remaining gap to the
ACT/DVE busy floor is ~6.5 us of pipeline fill (x-load latency + the
normalize->transpose->Sv chain) and ~3.5 us of drain (final evacuation +
split store + fixed DMA latency + barriers).
"""

import numpy as np

B = 2
T = 2048
D = 1024
H = 16
Dh = 64
N_CORES = 8
PAIRS = (B * H) // N_CORES  # 4 per core
P = 128  # SBUF partitions

_COMPILED = {}

# test-harness knobs (default off; harness calls kernel() with these untouched)
TRACE = False
MM_DTYPE = "float32r"
LAST_RESULT = None


class _Balancer:
    """Static greedy assignment of PSUM-evacuation copies (and other movable
    elementwise work) to ACT/DVE/Pool using the v1 cost model's rates."""

    def __init__(self, nc):
        self.nc = nc
        self.busy = {"act": 0.0, "dve": 0.0, "pool": 0.0}

    def add_fixed(self, eng, ns):
        self.busy[eng] += ns

    @staticmethod
    def _cost(eng, free, psum_src, dve_2x):
        if eng == "act":
            return free * 0.8333 + 185.0
        if eng == "dve":
            mult = 0.5 if dve_2x else 1.0
            init = 125.0 if psum_src else 60.0
            return free * 1.0417 * mult + init
        return free * 0.8333  # pool

    def copy(self, out, in_, free, psum_src=True, dve_2x=False, engines=("act", "dve")):
        best = min(engines, key=lambda e: self.busy[e] + self._cost(e, free, psum_src, dve_2x))
        c = self._cost(best, free, psum_src, dve_2x)
        self.busy[best] += c
        nc = self.nc
        if best == "act":
            nc.scalar.copy(out, in_)
        elif best == "dve":
            nc.vector.tensor_copy(out, in_)
        else:
            nc.gpsimd.tensor_copy(out, in_)
        return best


def _build_nc(pairs=PAIRS, t=T, mm_dtype_name="float32r", spill=True, repeat=1):
    _import_concourse()
    from contextlib import ExitStack

    import concourse.bass as bass
    import concourse.tile as tile
    from concourse import mybir

    f32 = mybir.dt.float32
    bf16 = mybir.dt.bfloat16
    nt = t // P  # m-blocks per pair (16)
    ng = t // 512  # 512-wide col groups (4)
    W = 512

    nc = bass.Bass()
    x_in = nc.declare_dram_parameter("x", [pairs, t, Dh], f32, isOutput=False)
    s_in = nc.declare_dram_parameter("s", [pairs, Dh, Dh], f32, isOutput=False)
    id_in = nc.declare_dram_parameter("ident", [P, P], f32, isOutput=False)
    out_d = nc.declare_dram_parameter("out", [pairs, t, t], f32, isOutput=True)

    with ExitStack() as ctx:
        tc = ctx.enter_context(tile.TileContext(nc))
        const_pool = ctx.enter_context(tc.tile_pool(name="const", bufs=1))
        v_pool = ctx.enter_context(tc.tile_pool(name="v", bufs=2))
        n_pool = ctx.enter_context(tc.tile_pool(name="norm", bufs=2))
        vt_pool = ctx.enter_context(tc.tile_pool(name="vt", bufs=3))
        ob_pool = ctx.enter_context(tc.tile_pool(name="outb", bufs=4))
        psw_pool = ctx.enter_context(tc.tile_pool(name="psw", bufs=4, space="PSUM"))

        bal = _Balancer(nc)
        consts = {}

        def emit_consts():
            # identity (bf16, for PE transposes), S -> bf16, warmups
            id_dma = const_pool.tile([P, P], f32)
            nc.scalar.dma_start(out=id_dma, in_=id_in[:, :])
            id16 = const_pool.tile([P, P], bf16)
            nc.gpsimd.tensor_copy(id16, id_dma)
            bal.add_fixed("pool", 107.0)
            # all pairs' S as bf16 via a single gpsimd cast DMA
            s16 = const_pool.tile([Dh, pairs, Dh], bf16)
            nc.gpsimd.dma_start(
                out=s16, in_=s_in[:, :, :].rearrange("p d e -> d p e")
            )
            bal.add_fixed("pool", 500.0)
            # ACT table warm (sqrt_and_others holds both Sqrt and Copy)
            act_warm = const_pool.tile([1, 1], f32)
            nc.scalar.activation(
                act_warm, id_dma[:1, :1], mybir.ActivationFunctionType.Sqrt
            )
            bal.add_fixed("act", 1500.0)
            # PE p-state pre-warm: ~3us of dummy matmuls on one slot so the
            # first real transposes/matmuls run at the full 2.4 GHz p-state
            ps_warm = psw_pool.tile([P, 1024], f32, tag="psw", name="ps_warm")
            nc.tensor.matmul(
                ps_warm[:1, :1],
                lhsT=id16[:1, :1],
                rhs=id16[:1, :1],
                start=True,
                stop=True,
            )
            consts["id16"] = id16
            consts["s16"] = s16

        # ---------- per-pair phase A: load + normalize + transpose + Sv ----
        state = {}

        def emit_A_load(p, half, first=False):
            """half 0: chunks 0-1, half 1: chunks 2-3 (each chunk = 512 rows)."""
            if half == 0:
                state[p] = {
                    "v": v_pool.tile([P, nt, Dh], f32, tag="v", name="v_sb"),
                    "v16": n_pool.tile([P, nt, Dh], bf16, tag="v16", name="v16"),
                    "sq": n_pool.tile([P, nt, Dh], bf16, tag="sq", name="sq16"),
                    "ss": n_pool.tile([P, nt], f32, tag="ss", name="ss"),
                    "nrm": n_pool.tile([P, nt], f32, tag="nrm", name="nrm"),
                    "rinv": n_pool.tile([P, nt], f32, tag="rinv", name="rinv"),
                    "rinv16": n_pool.tile([P, nt], bf16, tag="rinv16", name="rinv16"),
                    "vh": n_pool.tile([P, nt, Dh], bf16, tag="vh", name="vh16"),
                    "t32": n_pool.tile([P, nt, 32], f32, tag="t32", name="t32"),
                    "t16": n_pool.tile([P, nt, 16], f32, tag="t16", name="t16"),
                    "t8": n_pool.tile([P, nt, 8], f32, tag="t8", name="t8"),
                    "t4": n_pool.tile([P, nt, 4], f32, tag="t4", name="t4"),
                    "t2": n_pool.tile([P, nt, 2], f32, tag="t2", name="t2"),
                    "vt": vt_pool.tile([P, t], bf16, tag="vt", name="vt16"),
                    "svt": vt_pool.tile([P, t // 2], bf16, tag="svt", name="svt16"),
                }
            st = state[p]
            gn = nt // ng  # n-tiles per 512-row chunk (4)
            for g in (0, 1) if half == 0 else (2, 3):
                sl = slice(g * gn, (g + 1) * gn)
                # pair 0 is the pipeline fill: spread chunk loads over queues
                ld = (nc.sync, nc.scalar, nc.gpsimd, nc.sync)[g] if first else nc.sync
                ld.dma_start(
                    out=st["v"][:, sl, :],
                    in_=x_in[p][g * 512 : (g + 1) * 512, :].rearrange(
                        "(n p) d -> p n d", p=P
                    ),
                )
                for ssl in (sl,):
                    # cast + square + pairwise-tree row-sum, all on gpsimd
                    # (SBUF-only, legal there; keeps ACT/DVE for PSUM evac)
                    nc.gpsimd.tensor_copy(st["v16"][:, ssl, :], st["v"][:, ssl, :])
                    nc.gpsimd.tensor_mul(
                        st["sq"][:, ssl, :], st["v16"][:, ssl, :], st["v16"][:, ssl, :]
                    )
                    bal.add_fixed("pool", 2 * 256 * 0.8333)
                    srcs = st["sq"][:, ssl, :]
                    for lvl, wdt in enumerate((32, 16, 8, 4, 2)):
                        dst = st[f"t{wdt}"][:, ssl, :]
                        nc.gpsimd.tensor_add(dst, srcs[:, :, :wdt], srcs[:, :, wdt:])
                        bal.add_fixed("pool", gn * wdt * 0.8333)
                        srcs = dst
                    nc.gpsimd.tensor_add(
                        st["ss"][:, ssl], srcs[:, :, 0], srcs[:, :, 1]
                    )
                    bal.add_fixed("pool", gn * 0.8333)
                    # sqrt/recip/cast: per-chunk only for the fill pair
                    # (unblocks its transposes early); otherwise one batched
                    # op per half to cut ACT/DVE per-inst overhead
                    if first:
                        bs = ssl
                    elif g % 2 == 1:
                        bs = slice(ssl.stop - 2 * gn, ssl.stop)
                    else:
                        continue
                    nc.scalar.activation(
                        st["nrm"][:, bs], st["ss"][:, bs], mybir.ActivationFunctionType.Sqrt
                    )
                    bal.add_fixed("act", gn * 0.8333 + 185)
                    nc.vector.reciprocal(st["rinv"][:, bs], st["nrm"][:, bs])
                    bal.add_fixed("dve", 70)
                    nc.gpsimd.tensor_copy(st["rinv16"][:, bs], st["rinv"][:, bs])
                    bal.add_fixed("pool", 65)

        def emit_A_tr(p, phase, first=False):
            """phase 0: normalize + transpose all 16 n-tiles into a [64, 2048]
            bf16 psum view, ONE 2x-DVE evacuation into vt2[0:64], Pool then
            duplicates it onto partitions 64-127 (so wedge rhs can match any
            lhsT partition base).  phase 1: 4 Sv matmuls packed into one
            [128, 1024] f32 psum slot (partition half = t-half, col half =
            group parity), ONE evacuation into the packed svt2."""
            st = state[p]
            gn = nt // ng
            if phase == 0:
                ps = psw_pool.tile([P, 1024], f32, tag="psw", name="ps_a")
                ps_vt = ps.bitcast(bf16)[:Dh, :]
                for g in range(ng):
                    sl = slice(g * gn, (g + 1) * gn)
                    rb = st["rinv16"][:, sl].unsqueeze(-1).broadcast_to((P, gn, Dh))
                    nc.gpsimd.tensor_mul(st["vh"][:, sl, :], st["v16"][:, sl, :], rb)
                    bal.add_fixed("pool", gn * 64 * 0.8333)
                    for j in range(gn):
                        n = g * gn + j
                        nc.tensor.transpose(
                            ps_vt[:, g * W + j * P : g * W + (j + 1) * P],
                            st["vh"][:, n, :],
                            consts["id16"],
                        )
                    if first and g == 1:
                        # fill: evacuate the first column-half early so the
                        # Sv chain starts ~2 us sooner
                        bal.copy(st["vt"][:Dh, :1024], ps_vt[:, :1024], 1024, psum_src=True, dve_2x=True)
                if first:
                    bal.copy(st["vt"][:Dh, 1024:], ps_vt[:, 1024:], 1024, psum_src=True, dve_2x=True)
                else:
                    bal.copy(st["vt"][:Dh, :], ps_vt, 2048, psum_src=True, dve_2x=True)
                nc.gpsimd.tensor_copy(st["vt"][Dh:, :], st["vt"][:Dh, :])
                bal.add_fixed("pool", 2048 * 0.8333)
            else:
                ps_sv = psw_pool.tile([P, 1024], f32, tag="psw", name="ps_sv")
                for g in range(ng):
                    nc.tensor.matmul(
                        ps_sv[(g // 2) * Dh : (g // 2 + 1) * Dh, (g % 2) * W : (g % 2 + 1) * W],
                        lhsT=consts["s16"][:, p, :],
                        rhs=st["vt"][:Dh, g * W : (g + 1) * W],
                        start=True,
                        stop=True,
                    )
                    if first and g == 1:
                        # fill: B(h=0) only needs SvT's first t-half
                        bal.copy(st["svt"][:Dh, :], ps_sv[:Dh, :], 1024, psum_src=True)
                if first:
                    bal.copy(st["svt"][Dh:, :], ps_sv[Dh:, :], 1024, psum_src=True)
                else:
                    bal.copy(st["svt"][:, :], ps_sv, 1024, psum_src=True)

        # ------ per-pair phase B: one (row-half, 512-col group) sub-block --
        # Row interleave within a half: t = h*1024 + q*8 + m, so the wedge
        # m-block of half h uses lhsT = svt[:, h*1024 + m : h*1024+1024 : 8]
        # (only that half of SvT -> half-barrier on phase A).
        nh = nt // 2  # m-chunks per half (8)

        def emit_B_block(p, h, g, last=False):
            st = state[p]
            ob = ob_pool.tile([P, nh, 516], f32, tag="ob", name="ob")
            rhs = st["vt"][h * Dh : (h + 1) * Dh, g * W : (g + 1) * W]
            for mm in range(0, nh, 2):
                ps_w = psw_pool.tile([P, 1024], f32, tag="psw", name="ps_w")
                for ms in range(2):
                    m = mm + ms
                    nc.tensor.matmul(
                        ps_w[:, ms * W : (ms + 1) * W],
                        lhsT=st["svt"][h * Dh : (h + 1) * Dh, m : 1024 : nh],
                        rhs=rhs,
                        start=True,
                        stop=True,
                    )
                if last and mm == nh - 2:
                    # drain: ACT's m6 copy runs during matmul m7 (slice deps),
                    # so split m7's evacuation so neither engine gates long
                    nc.scalar.copy(ob[:, mm, :W], ps_w[:, :W])
                    nc.scalar.copy(ob[:, mm + 1, :256], ps_w[:, W : W + 256])
                    nc.vector.tensor_copy(ob[:, mm + 1, 256:W], ps_w[:, W + 256 :])
                else:
                    bal.copy(ob[:, mm : mm + 2, :W], ps_w, 1024, psum_src=True)
            if last:
                # drain: split the final store across two queues so the
                # tail transfer+latency halves
                nc.sync.dma_start(
                    out=out_d[p][h * 1024 : (h + 1) * 1024, g * W : g * W + 256],
                    in_=ob[:, :, :256],
                )
                nc.gpsimd.dma_start(
                    out=out_d[p][h * 1024 : (h + 1) * 1024, g * W + 256 : (g + 1) * W],
                    in_=ob[:, :, 256:W],
                )
            else:
                # flat store: DRAM-side AP balances to [[rows,1024],[1,1],[1,W]]
                nc.sync.dma_start(
                    out=out_d[p][h * 1024 : (h + 1) * 1024, g * W : (g + 1) * W],
                    in_=ob[:, :, :W],
                )

        # ---------- emission with cross-pair software pipelining -----------
        plist = [q for _ in range(repeat) for q in range(pairs)]

        def emit_A_slice(p, i, first=False):
            if i == 0:
                emit_A_load(p, 0, first=first)
            elif i == 1:
                emit_A_load(p, 1, first=first)
            elif i == 2:
                emit_A_tr(p, 0)
            else:
                emit_A_tr(p, 1)

        emit_A_slice(plist[0], 0, first=True)
        emit_consts()
        emit_A_slice(plist[0], 1, first=True)
        emit_A_tr(plist[0], 0, first=True)
        emit_A_tr(plist[0], 1)
        for idx, p in enumerate(plist):
            nxt = plist[idx + 1] if idx + 1 < len(plist) else None
            for h in range(2):
                for g in range(ng):
                    emit_B_block(p, h, g, last=(nxt is None and h == 1 and g == ng - 1))
                    if nxt is not None and h == 0:
                        emit_A_slice(nxt, g)

    if spill:
        _spill_waits(nc)
    return nc


def _spill_waits(nc, multi_ok=("EventSemaphore",), max_keep=1):
    """Walrus encodes at most one sync-wait on Matmult (embedded weight load)
    and DMACopy; move extra waits onto a preceding same-engine EventSemaphore
    (which supports many waits). The engine sequencer processes instructions
    in order, so a preceding wait is semantically identical."""
    from concourse import mybir

    n_spilled = 0
    for f in nc.m.functions:
        for bb in f.blocks:
            il = bb.instructions
            out = []
            for inst in il:
                si = getattr(inst, "sync_info", None)
                waits = list((si.on_wait if si else None) or [])
                cap = 2 if inst.opcode in multi_ok else max_keep
                if len(waits) > cap:
                    moved, keep = waits[:-max_keep], waits[-max_keep:]
                    for k in range(0, len(moved), 2):
                        es = mybir.InstEventSemaphore(
                            name=f"{inst.name}-wspill{k}",
                            engine=inst.engine,
                            ins=[],
                            outs=[],
                            sync_info=mybir.SyncInfo(
                                on_wait=moved[k : k + 2], on_update=[]
                            ),
                        )
                        out.append(es)
                    inst.sync_info = mybir.SyncInfo(
                        on_wait=keep, on_update=list(si.on_update or [])
                    )
                    n_spilled += 1
                out.append(inst)
            il[:] = out
    return n_spilled


def _import_concourse():
    try:
        import concourse  # noqa: F401
    except ImportError:
        import sys

        for p in ("/opt/trn_rl_repo", "/root/.axon_site/_ro/trn_rl_repo"):
            if p not in sys.path:
                sys.path.insert(0, p)


def _ensure_device_backend():
    """If the process pinned JAX_PLATFORMS to cpu, lift the pin so the
    NeuronCores (axon platform) are reachable for the kernel run."""
    import os

    plats = os.environ.get("JAX_PLATFORMS", "")
    if plats and "axon" not in plats and "neuron" not in plats:
        os.environ["JAX_PLATFORMS"] = ""
        try:
            import jax

            jax.extend.backend.clear_backends()
        except Exception:
            pass


def kernel(x, A, window_size=None):
    _import_concourse()
    _ensure_device_backend()
    from concourse.bass_utils import run_bass_kernel_spmd

    x = np.ascontiguousarray(x, dtype=np.float32)
    A = np.ascontiguousarray(A, dtype=np.float32)
    assert x.shape == (B, T, D) and A.shape == (H, Dh, Dh)

    nc = _COMPILED.get(MM_DTYPE)
    if nc is None:
        nc = _build_nc(mm_dtype_name=MM_DTYPE)
        _COMPILED[MM_DTYPE] = nc

    # x[b, t, h*64:(h+1)*64] per (b,h) pair; pair index bh = b*H + h.
    xv = x.reshape(B, T, H, Dh).transpose(0, 2, 1, 3).reshape(B * H, T, Dh)
    S = (A - np.swapaxes(A, -1, -2)).astype(np.float32)  # replicated with heads
    S_all = np.tile(S, (B, 1, 1))
    ident = np.eye(P, dtype=np.float32)
    in_maps = []
    for c in range(N_CORES):
        sl = slice(c * PAIRS, (c + 1) * PAIRS)
        in_maps.append(
            {
                "x": np.ascontiguousarray(xv[sl]),
                "s": np.ascontiguousarray(S_all[sl]),
                "ident": ident,
            }
        )
    res = run_bass_kernel_spmd(nc, in_maps, list(range(N_CORES)), trace=TRACE)
    global LAST_RESULT
    LAST_RESULT = res
    outs = [res.results[c]["out"] for c in range(N_CORES)]
    full = np.concatenate(outs, axis=0).reshape(B, H, T, T)
    return full


# revision 79
# speedup vs baseline: 1.0037x; 1.0037x over previous
"""Trainium2 Bass kernel for nn_DirectionalWedgeBias.

Computes, per (batch b, head h):
    v      = x[b].reshape(T, H, Dh)[:, h, :]          # [T, Dh]
    v_hat  = v / max(||v||_2, eps)  (row-wise)
    S      = A[h] - A[h]^T                            # [Dh, Dh]
    wedge  = (v_hat @ S) @ v_hat^T                    # [T, T]

Full shapes: x [2, 2048, 1024] f32, A [16, 64, 64] f32 -> out [2, 16, 2048, 2048] f32.

Sharding: 32 independent (b, h) pairs split 4-per-core across 8 NeuronCores
(data + head parallel; the tiny skew-symmetric S is replicated/sliced with the
heads). Host pre-slices x into per-core [4, T, Dh] blocks, forms S = A - A^T,
and re-stacks the per-core [4, T, T] results.

Per-core dataflow (Tile framework), designed around the v1 CoreSim cost model
and the walrus BIR constraints:

  - All wedge math runs in bf16 (v_hat, S, SvT; rel err ~4.7e-3 vs the 2e-2
    budget): bf16 matmuls/transposes run 1 cyc/row on the PE and bf16 SBUF
    elementwise ops hit the DVE/engine fast paths.
  - Interleaved m-blocks: the wedge row-block m of row-half ht uses
    lhsT = svt[:, m::8] within that half, so PSUM partition q of block m
    holds output row t = 1024*ht + q*8 + m.  A (half, col-group) sub-block
    stages [128 part, 8, 512] in SBUF whose flat (partition, chunk, col)
    order IS row-major DRAM order: the store's DRAM-side AP balances to
    [[rows, 1024], [1, 1], [1, 512]], whose free size (and hence DMA queue
    cost) is 512 elems -> ~0.8 us per 2 MiB store instead of the ~25 us a
    [128, ...]-leading AP would cost.  The staging tile keeps a 516-elem
    chunk stride so its free dims cannot re-merge during AP balancing.
  - PSUM->SBUF evacuation is the critical path: walrus forbids GPSIMD from
    touching PSUM, so only ACT (0.83 ns/elem + 185 ns/inst) and DVE
    (1.04 ns/elem + 125 ns/inst; 0.52 for all-2-byte ops) can drain the
    64 MiB of wedge PSUM.  A static greedy balancer splits every
    evacuation copy between them (~1.85 elems/ns combined -> ~80 us busy
    each; the PE's 61 us of matmuls hides under it).
  - Everything SBUF-only rides the otherwise-idle gpsimd queue: the x->bf16
    cast, the square, a pairwise-tree row-sum (replacing the DVE-only
    reduce), the 1/||v|| broadcast multiply, and the vT partition-half
    duplication.  SP carries all input loads and output stores.
  - A-phase packing: all 16 transposes of a pair write one [64, 2048] bf16
    PSUM view (one 2x DVE evacuation), gpsimd then mirrors vT onto
    partitions 64-127 so every wedge matmul finds lhsT and rhs on the same
    partition base; the 4 Sv matmuls pack into one [128, 1024] f32 slot
    (partition half = t-half, column half = group parity) for a single
    evacuation into the packed SvT.
  - PSUM: one shared 4-slot ring of [128, 1024] (2 banks each) serves wedge
    tiles, the transpose view and the Sv pack; the PE runs up to ~1.7 us
    ahead of the evacuations.  PE p-state: a warmup matmul starts the ramp
    clock during the fill (pe_busy_start never resets on the sub-us stalls
    this schedule produces, verified against the cost model).
  - Software pipelining: pair p+1's load/normalize (2 slices) and
    transpose/Sv (2 slices) are emitted between pair p's first four wedge
    sub-blocks; the drain splits the final evacuation ACT/DVE.
  - walrus encodes at most ONE semaphore wait on most instructions (and two
    on EventSemaphore), so `_spill_waits` post-processes the Tile-scheduled
    BIR, hoisting excess waits onto preceding same-engine EventSemaphores
    (sequencers run in order, so this is semantics-preserving).

Cost-model (CoreSim) per-core time: ~88.9 us vs the 121.5 us baseline
(engine busy: ACT ~79 us, DVE ~78 us, PE ~61 us, SP ~36 us, gpsimd ~22 us;
the fill pair additionally stages its vT/SvT evacuations per half inside the
same PSUM slot acquisitions so the first wedge block starts ~2 us sooner);
verified on the 8 NeuronCores at rel err 4.73e-3.  The remaining gap to the
ACT/DVE busy floor is ~6.5 us of pipeline fill (x-load latency + the
normalize->transpose->Sv chain) and ~3.5 us of drain (final evacuation +
split store + fixed DMA latency + barriers).
"""

import numpy as np

B = 2
T = 2048
D = 1024
H = 16
Dh = 64
N_CORES = 8
PAIRS = (B * H) // N_CORES  # 4 per core
P = 128  # SBUF partitions

_COMPILED = {}

# test-harness knobs (default off; harness calls kernel() with these untouched)
TRACE = False
MM_DTYPE = "float32r"
LAST_RESULT = None


class _Balancer:
    """Static greedy assignment of PSUM-evacuation copies (and other movable
    elementwise work) to ACT/DVE/Pool using the v1 cost model's rates."""

    def __init__(self, nc):
        self.nc = nc
        self.busy = {"act": 0.0, "dve": 0.0, "pool": 0.0}

    def add_fixed(self, eng, ns):
        self.busy[eng] += ns

    @staticmethod
    def _cost(eng, free, psum_src, dve_2x):
        if eng == "act":
            return free * 0.8333 + 185.0
        if eng == "dve":
            mult = 0.5 if dve_2x else 1.0
            init = 125.0 if psum_src else 60.0
            return free * 1.0417 * mult + init
        return free * 0.8333  # pool

    def copy(self, out, in_, free, psum_src=True, dve_2x=False, engines=("act", "dve")):
        best = min(engines, key=lambda e: self.busy[e] + self._cost(e, free, psum_src, dve_2x))
        c = self._cost(best, free, psum_src, dve_2x)
        self.busy[best] += c
        nc = self.nc
        if best == "act":
            nc.scalar.copy(out, in_)
        elif best == "dve":
            nc.vector.tensor_copy(out, in_)
        else:
            nc.gpsimd.tensor_copy(out, in_)
        return best


def _build_nc(pairs=PAIRS, t=T, mm_dtype_name="float32r", spill=True, repeat=1):
    _import_concourse()
    from contextlib import ExitStack

    import concourse.bass as bass
    import concourse.tile as tile
    from concourse import mybir

    f32 = mybir.dt.float32
    bf16 = mybir.dt.bfloat16
    nt = t // P  # m-blocks per pair (16)
    ng = t // 512  # 512-wide col groups (4)
    W = 512

    nc = bass.Bass()
    x_in = nc.declare_dram_parameter("x", [pairs, t, Dh], f32, isOutput=False)
    s_in = nc.declare_dram_parameter("s", [pairs, Dh, Dh], f32, isOutput=False)
    id_in = nc.declare_dram_parameter("ident", [P, P], f32, isOutput=False)
    out_d = nc.declare_dram_parameter("out", [pairs, t, t], f32, isOutput=True)

    with ExitStack() as ctx:
        tc = ctx.enter_context(tile.TileContext(nc))
        const_pool = ctx.enter_context(tc.tile_pool(name="const", bufs=1))
        v_pool = ctx.enter_context(tc.tile_pool(name="v", bufs=2))
        n_pool = ctx.enter_context(tc.tile_pool(name="norm", bufs=2))
        vt_pool = ctx.enter_context(tc.tile_pool(name="vt", bufs=3))
        ob_pool = ctx.enter_context(tc.tile_pool(name="outb", bufs=4))
        psw_pool = ctx.enter_context(tc.tile_pool(name="psw", bufs=4, space="PSUM"))

        bal = _Balancer(nc)
        consts = {}

        def emit_consts():
            # identity (bf16, for PE transposes), S -> bf16, warmups
            id_dma = const_pool.tile([P, P], f32)
            nc.scalar.dma_start(out=id_dma, in_=id_in[:, :])
            id16 = const_pool.tile([P, P], bf16)
            nc.gpsimd.tensor_copy(id16, id_dma)
            bal.add_fixed("pool", 107.0)
            # all pairs' S as bf16 via a single gpsimd cast DMA
            s16 = const_pool.tile([Dh, pairs, Dh], bf16)
            nc.gpsimd.dma_start(
                out=s16, in_=s_in[:, :, :].rearrange("p d e -> d p e")
            )
            bal.add_fixed("pool", 500.0)
            # ACT table warm (sqrt_and_others holds both Sqrt and Copy)
            act_warm = const_pool.tile([1, 1], f32)
            nc.scalar.activation(
                act_warm, id_dma[:1, :1], mybir.ActivationFunctionType.Sqrt
            )
            bal.add_fixed("act", 1500.0)
            # PE p-state pre-warm: ~3us of dummy matmuls on one slot so the
            # first real transposes/matmuls run at the full 2.4 GHz p-state
            ps_warm = psw_pool.tile([P, 1024], f32, tag="psw", name="ps_warm")
            nc.tensor.matmul(
                ps_warm[:1, :1],
                lhsT=id16[:1, :1],
                rhs=id16[:1, :1],
                start=True,
                stop=True,
            )
            consts["id16"] = id16
            consts["s16"] = s16

        # ---------- per-pair phase A: load + normalize + transpose + Sv ----
        state = {}

        def emit_A_load(p, half, first=False):
            """half 0: chunks 0-1, half 1: chunks 2-3 (each chunk = 512 rows)."""
            if half == 0:
                state[p] = {
                    "v": v_pool.tile([P, nt, Dh], f32, tag="v", name="v_sb"),
                    "v16": n_pool.tile([P, nt, Dh], bf16, tag="v16", name="v16"),
                    "sq": n_pool.tile([P, nt, Dh], bf16, tag="sq", name="sq16"),
                    "ss": n_pool.tile([P, nt], f32, tag="ss", name="ss"),
                    "nrm": n_pool.tile([P, nt], f32, tag="nrm", name="nrm"),
                    "rinv": n_pool.tile([P, nt], f32, tag="rinv", name="rinv"),
                    "rinv16": n_pool.tile([P, nt], bf16, tag="rinv16", name="rinv16"),
                    "vh": n_pool.tile([P, nt, Dh], bf16, tag="vh", name="vh16"),
                    "t32": n_pool.tile([P, nt, 32], f32, tag="t32", name="t32"),
                    "t16": n_pool.tile([P, nt, 16], f32, tag="t16", name="t16"),
                    "t8": n_pool.tile([P, nt, 8], f32, tag="t8", name="t8"),
                    "t4": n_pool.tile([P, nt, 4], f32, tag="t4", name="t4"),
                    "t2": n_pool.tile([P, nt, 2], f32, tag="t2", name="t2"),
                    "vt": vt_pool.tile([P, t], bf16, tag="vt", name="vt16"),
                    "svt": vt_pool.tile([P, t // 2], bf16, tag="svt", name="svt16"),
                }
            st = state[p]
            gn = nt // ng  # n-tiles per 512-row chunk (4)
            for g in (0, 1) if half == 0 else (2, 3):
                sl = slice(g * gn, (g + 1) * gn)
                # pair 0 is the pipeline fill: spread chunk loads over queues
                ld = (nc.sync, nc.scalar, nc.gpsimd, nc.sync)[g] if first else nc.sync
                ld.dma_start(
                    out=st["v"][:, sl, :],
                    in_=x_in[p][g * 512 : (g + 1) * 512, :].rearrange(
                        "(n p) d -> p n d", p=P
                    ),
                )
                for ssl in (sl,):
                    # cast + square + pairwise-tree row-sum, all on gpsimd
                    # (SBUF-only, legal there; keeps ACT/DVE for PSUM evac)
                    nc.gpsimd.tensor_copy(st["v16"][:, ssl, :], st["v"][:, ssl, :])
                    nc.gpsimd.tensor_mul(
                        st["sq"][:, ssl, :], st["v16"][:, ssl, :], st["v16"][:, ssl, :]
                    )
                    bal.add_fixed("pool", 2 * 256 * 0.8333)
                    srcs = st["sq"][:, ssl, :]
                    for lvl, wdt in enumerate((32, 16, 8, 4, 2)):
                        dst = st[f"t{wdt}"][:, ssl, :]
                        nc.gpsimd.tensor_add(dst, srcs[:, :, :wdt], srcs[:, :, wdt:])
                        bal.add_fixed("pool", gn * wdt * 0.8333)
                        srcs = dst
                    nc.gpsimd.tensor_add(
                        st["ss"][:, ssl], srcs[:, :, 0], srcs[:, :, 1]
                    )
                    bal.add_fixed("pool", gn * 0.8333)
                    # sqrt/recip/cast: per-chunk only for the fill pair
                    # (unblocks its transposes early); otherwise one batched
                    # op per half to cut ACT/DVE per-inst overhead
                    if first:
                        bs = ssl
                    elif g % 2 == 1:
                        bs = slice(ssl.stop - 2 * gn, ssl.stop)
                    else:
                        continue
                    nc.scalar.activation(
                        st["nrm"][:, bs], st["ss"][:, bs], mybir.ActivationFunctionType.Sqrt
                    )
                    bal.add_fixed("act", gn * 0.8333 + 185)
                    nc.vector.reciprocal(st["rinv"][:, bs], st["nrm"][:, bs])
                    bal.add_fixed("dve", 70)
                    nc.gpsimd.tensor_copy(st["rinv16"][:, bs], st["rinv"][:, bs])
                    bal.add_fixed("pool", 65)

        def emit_A_tr(p, phase, first=False):
            """phase 0: normalize + transpose all 16 n-tiles into a [64, 2048]
            bf16 psum view, ONE 2x-DVE evacuation into vt2[0:64], Pool then
            duplicates it onto partitions 64-127 (so wedge rhs can match any
            lhsT partition base).  phase 1: 4 Sv matmuls packed into one
            [128, 1024] f32 psum slot (partition half = t-half, col half =
            group parity), ONE evacuation into the packed svt2."""
            st = state[p]
            gn = nt // ng
            if phase == 0:
                ps = psw_pool.tile([P, 1024], f32, tag="psw", name="ps_a")
                ps_vt = ps.bitcast(bf16)[:Dh, :]
                for g in range(ng):
                    sl = slice(g * gn, (g + 1) * gn)
                    rb = st["rinv16"][:, sl].unsqueeze(-1).broadcast_to((P, gn, Dh))
                    nc.gpsimd.tensor_mul(st["vh"][:, sl, :], st["v16"][:, sl, :], rb)
                    bal.add_fixed("pool", gn * 64 * 0.8333)
                    for j in range(gn):
                        n = g * gn + j
                        nc.tensor.transpose(
                            ps_vt[:, g * W + j * P : g * W + (j + 1) * P],
                            st["vh"][:, n, :],
                            consts["id16"],
                        )
                    if first and g == 1:
                        # fill: evacuate the first column-half early so the
                        # Sv chain starts ~2 us sooner
                        bal.copy(st["vt"][:Dh, :1024], ps_vt[:, :1024], 1024, psum_src=True, dve_2x=True)
                if first:
                    bal.copy(st["vt"][:Dh, 1024:], ps_vt[:, 1024:], 1024, psum_src=True, dve_2x=True)
                else:
                    bal.copy(st["vt"][:Dh, :], ps_vt, 2048, psum_src=True, dve_2x=True)
                nc.gpsimd.tensor_copy(st["vt"][Dh:, :], st["vt"][:Dh, :])
                bal.add_fixed("pool", 2048 * 0.8333)
            else:
                ps_sv = psw_pool.tile([P, 1024], f32, tag="psw", name="ps_sv")
                for g in range(ng):
                    nc.tensor.matmul(
                        ps_sv[(g // 2) * Dh : (g // 2 + 1) * Dh, (g % 2) * W : (g % 2 + 1) * W],
                        lhsT=consts["s16"][:, p, :],
                        rhs=st["vt"][:Dh, g * W : (g + 1) * W],
                        start=True,
                        stop=True,
                    )
                    if first and g == 1:
                        # fill: B(h=0) only needs SvT's first t-half
                        bal.copy(st["svt"][:Dh, :], ps_sv[:Dh, :], 1024, psum_src=True)
                if first:
                    bal.copy(st["svt"][Dh:, :], ps_sv[Dh:, :], 1024, psum_src=True)
                else:
                    bal.copy(st["svt"][:, :], ps_sv, 1024, psum_src=True)

        # ------ per-pair phase B: one (row-half, 512-col group) sub-block --
        # Row interleave within a half: t = h*1024 + q*8 + m, so the wedge
        # m-block of half h uses lhsT = svt[:, h*1024 + m : h*1024+1024 : 8]
        # (only that half of SvT -> half-barrier on phase A).
        nh = nt // 2  # m-chunks per half (8)

        def emit_B_block(p, h, g, last=False):
            st = state[p]
            ob = ob_pool.tile([P, nh, 516], f32, tag="ob", name="ob")
            rhs = st["vt"][h * Dh : (h + 1) * Dh, g * W : (g + 1) * W]
            for mm in range(0, nh, 2):
                ps_w = psw_pool.tile([P, 1024], f32, tag="psw", name="ps_w")
                for ms in range(2):
                    m = mm + ms
                    nc.tensor.matmul(
                        ps_w[:, ms * W : (ms + 1) * W],
                        lhsT=st["svt"][h * Dh : (h + 1) * Dh, m : 1024 : nh],
                        rhs=rhs,
                        start=True,
                        stop=True,
                    )
                if last and mm == nh - 2:
                    # drain: split the final evacuation across ACT and DVE
                    nc.scalar.copy(ob[:, mm, :W], ps_w[:, :W])
                    nc.vector.tensor_copy(ob[:, mm + 1, :W], ps_w[:, W:])
                else:
                    bal.copy(ob[:, mm : mm + 2, :W], ps_w, 1024, psum_src=True)
            if last:
                # drain: split the final store across two queues so the
                # tail transfer+latency halves
                nc.sync.dma_start(
                    out=out_d[p][h * 1024 : (h + 1) * 1024, g * W : g * W + 256],
                    in_=ob[:, :, :256],
                )
                nc.gpsimd.dma_start(
                    out=out_d[p][h * 1024 : (h + 1) * 1024, g * W + 256 : (g + 1) * W],
                    in_=ob[:, :, 256:W],
                )
            else:
                # flat store: DRAM-side AP balances to [[rows,1024],[1,1],[1,W]]
                nc.sync.dma_start(
                    out=out_d[p][h * 1024 : (h + 1) * 1024, g * W : (g + 1) * W],
                    in_=ob[:, :, :W],
                )

        # ---------- emission with cross-pair software pipelining -----------
        plist = [q for _ in range(repeat) for q in range(pairs)]

        def emit_A_slice(p, i, first=False):
            if i == 0:
                emit_A_load(p, 0, first=first)
            elif i == 1:
                emit_A_load(p, 1, first=first)
            elif i == 2:
                emit_A_tr(p, 0)
            else:
                emit_A_tr(p, 1)

        emit_A_slice(plist[0], 0, first=True)
        emit_consts()
        emit_A_slice(plist[0], 1, first=True)
        emit_A_tr(plist[0], 0, first=True)
        emit_A_tr(plist[0], 1)
        for idx, p in enumerate(plist):
            nxt = plist[idx + 1] if idx + 1 < len(plist) else None
            for h in range(2):
                for g in range(ng):
                    emit_B_block(p, h, g, last=(nxt is None and h == 1 and g == ng - 1))
                    if nxt is not None and h == 0:
                        emit_A_slice(nxt, g)

    if spill:
        _spill_waits(nc)
    return nc


def _spill_waits(nc, multi_ok=("EventSemaphore",), max_keep=1):
    """Walrus encodes at most one sync-wait on Matmult (embedded weight load)
    and DMACopy; move extra waits onto a preceding same-engine EventSemaphore
    (which supports many waits). The engine sequencer processes instructions
    in order, so a preceding wait is semantically identical."""
    from concourse import mybir

    n_spilled = 0
    for f in nc.m.functions:
        for bb in f.blocks:
            il = bb.instructions
            out = []
            for inst in il:
                si = getattr(inst, "sync_info", None)
                waits = list((si.on_wait if si else None) or [])
                cap = 2 if inst.opcode in multi_ok else max_keep
                if len(waits) > cap:
                    moved, keep = waits[:-max_keep], waits[-max_keep:]
                    for k in range(0, len(moved), 2):
                        es = mybir.InstEventSemaphore(
                            name=f"{inst.name}-wspill{k}",
                            engine=inst.engine,
                            ins=[],
                            outs=[],
                            sync_info=mybir.SyncInfo(
                                on_wait=moved[k : k + 2], on_update=[]
                            ),
                        )
                        out.append(es)
                    inst.sync_info = mybir.SyncInfo(
                        on_wait=keep, on_update=list(si.on_update or [])
                    )
                    n_spilled += 1
                out.append(inst)
            il[:] = out
    return n_spilled


def _import_concourse():
    try:
        import concourse  # noqa: F401
    except ImportError:
        import sys

        for p in ("/opt/trn_rl_repo", "/root/.axon_site/_ro/trn_rl_repo"):
            if p not in sys.path:
                sys.path.insert(0, p)


def _ensure_device_backend():
    """If the process pinned JAX_PLATFORMS to cpu, lift the pin so the
    NeuronCores (axon platform) are reachable for the kernel run."""
    import os

    plats = os.environ.get("JAX_PLATFORMS", "")
    if plats and "axon" not in plats and "neuron" not in plats:
        os.environ["JAX_PLATFORMS"] = ""
        try:
            import jax

            jax.extend.backend.clear_backends()
        except Exception:
            pass


def kernel(x, A, window_size=None):
    _import_concourse()
    _ensure_device_backend()
    from concourse.bass_utils import run_bass_kernel_spmd

    x = np.ascontiguousarray(x, dtype=np.float32)
    A = np.ascontiguousarray(A, dtype=np.float32)
    assert x.shape == (B, T, D) and A.shape == (H, Dh, Dh)

    nc = _COMPILED.get(MM_DTYPE)
    if nc is None:
        nc = _build_nc(mm_dtype_name=MM_DTYPE)
        _COMPILED[MM_DTYPE] = nc

    # x[b, t, h*64:(h+1)*64] per (b,h) pair; pair index bh = b*H + h.
    xv = x.reshape(B, T, H, Dh).transpose(0, 2, 1, 3).reshape(B * H, T, Dh)
    S = (A - np.swapaxes(A, -1, -2)).astype(np.float32)  # replicated with heads
    S_all = np.tile(S, (B, 1, 1))
    ident = np.eye(P, dtype=np.float32)
    in_maps = []
    for c in range(N_CORES):
        sl = slice(c * PAIRS, (c + 1) * PAIRS)
        in_maps.append(
            {
                "x": np.ascontiguousarray(xv[sl]),
                "s": np.ascontiguousarray(S_all[sl]),
                "ident": ident,
            }
        )
    res = run_bass_kernel_spmd(nc, in_maps, list(range(N_CORES)), trace=TRACE)
    global LAST_RESULT
    LAST_RESULT = res
    outs = [res.results[c]["out"] for c in range(N_CORES)]
    full = np.concatenate(outs, axis=0).reshape(B, H, T, T)
    return full
